# revision 33
# baseline (speedup 1.0000x reference)
"""Trainium2 Bass kernel for nn_BSplineBasis (KAN-style B-spline layer).

reference:  t = tanh(x); basis = cubic B-spline basis of t over a uniform
            12-knot grid ([B, in, 8]); out = einsum('bin,ion->bo', basis,
            coeffs) + x @ base_weight;  kl = zeros(1).

Reformulation (exact in exact arithmetic): on t in [-1, 1] the 8 basis
functions span an 8-dim space of C^2 piecewise cubics with interior knots at
{-0.6,-0.2,0.2,0.6}. We represent them in a segmented clamped truncated-power
basis C(2,3): two segments starting at {-1.0, -0.2} covering {2, 3} grid
intervals; per segment, with v = clamp((t-s0)/h, 0, n), the channels are
{v, v^2, v^3} plus {relu(v-rr)^3 : rr=1..n-1}. That is 9 channels + a
constant. The constant folds to a per-output bias (added during the
PSUM->SBUF copy); the einsum + residual become one dense matmul over
K = (9+1)*512 = 5120 (9 spline channels + x), with weights host-folded from
coeffs/base_weight in float64.

All matmul operands are fp16 (measured 1.5e-3 rel L2 vs the fp32 reference;
channel values are O(1..27) and conversion coefficients are O(1), so fp16
rounding is not amplified). Accumulation is fp32 in PSUM.

Per core the pipeline processes batch-column groups (small first groups fill
the PE pipeline quickly): DMA x^T / x^T-fp16 -> tanh (Scalar engine, in
place) -> channels (Scalar relu + Vector clamps/multiplies, fp16) -> 40
accumulating matmuls per 128-row PSUM tile -> Vector copy+bias-add -> DMA.

Sharding: data-parallel over batch; each of the 8 cores handles 2048 rows.
Weights/bias are replicated (host-folded, ~5 MB fp16). x is pre-transposed
on host so the contraction dim lands on SBUF partitions.
"""

import numpy as np
from contextlib import ExitStack

import concourse.bass as bass
import concourse.mybir as mybir
import concourse.tile as tile
from concourse import bacc
from concourse.bass_utils import run_bass_kernel_spmd

F32 = mybir.dt.float32
F16 = mybir.dt.float16
AF = mybir.ActivationFunctionType
ALU = mybir.AluOpType

# ---------------- problem constants (hardcoded per harness contract) --------
B_TOT = 16384
IN_F = 512
OUT_F = 512
NUM_CTRL = 8
GRID_SIZE, ORDER = 5, 3
N_CORES = 8
B_CORE = B_TOT // N_CORES          # 2048
# batch-column group sizes per pipeline stage: small first groups fill the
# PE pipeline quickly; big groups amortize per-instruction overhead.
GROUPS = [128, 128, 128, 128, 512, 512, 512]
assert sum(GROUPS) == B_CORE
H = 2.0 / GRID_SIZE                # 0.4 knot spacing
SEGS = [(-1.0, 2), (-0.2, 3)]   # (start, n_intervals)
N_CH = 1 + sum(3 + (n - 1) for (_, n) in SEGS)   # x + 9 spline channels = 10
K_DIM = N_CH * IN_F                # 5120
N_CHUNK = K_DIM // 128             # 40
NIT = IN_F // 128                  # 4 i-chunks per channel

_cache = {}


# ---------------- host-side spline algebra ----------------------------------
def _bspline_basis_np(x, grid, order):
    xe = x[..., None]
    b = ((xe >= grid[:-1]) & (xe < grid[1:])).astype(x.dtype)
    for k in range(1, order + 1):
        ld = grid[k:-1] - grid[:-k - 1]
        ld = np.where(ld == 0, 1.0, ld)
        rd = grid[k + 1:] - grid[1:-k]
        rd = np.where(rd == 0, 1.0, rd)
        b = ((xe - grid[:-k - 1]) / ld) * b[..., :-1] \
            + ((grid[k + 1:] - xe) / rd) * b[..., 1:]
    return b


def _seg_channels_np(t):
    chans = []
    for (s0, n) in SEGS:
        v = np.clip((t - s0) / H, 0.0, float(n))
        chans += [v, v * v, v * v * v]
        for rr in range(1, n):
            r = np.maximum(v - float(rr), 0.0)
            chans.append(r * r * r)
    return chans


def _conversion():
    """conv[1+9, 8]: basis_n(t) = conv[0,n]*1 + sum_c conv[1+c,n]*chan_c(t)."""
    grid = np.linspace(-1.0 - ORDER * H, 1.0 + ORDER * H,
                       GRID_SIZE + 2 * ORDER + 1)
    ts = np.linspace(-1.0, 1.0, 20001)
    basis = _bspline_basis_np(ts, grid, ORDER)          # [S, 8]
    A = np.stack([np.ones_like(ts)] + _seg_channels_np(ts), axis=1)
    conv, _, _, _ = np.linalg.lstsq(A, basis, rcond=None)
    fit_err = np.abs(A @ conv - basis).max()
    assert fit_err < 1e-10, f"spline conversion fit failed: {fit_err}"
    return conv                                          # [10, 8]


def _fold_weights(coeffs, base_weight):
    """Returns W_dev [K_DIM, OUT_F] fp16 (channel-major rows) and bias [OUT_F] f32."""
    conv = _conversion()
    c64 = coeffs.astype(np.float64)
    bias = np.einsum('n,ion->o', conv[0], c64).astype(np.float32)
    W = np.empty((N_CH, IN_F, OUT_F), np.float64)
    W[0] = base_weight.astype(np.float64)
    for c in range(1, N_CH):
        W[c] = np.einsum('n,ion->io', conv[c], c64)
    return W.reshape(K_DIM, OUT_F).astype(np.float16), bias


# ---------------- device kernel ---------------------------------------------
def _build():
    nc = bacc.Bacc()
    xt = nc.dram_tensor("xt", [IN_F, B_CORE], F32, kind="ExternalInput")
    xt16 = nc.dram_tensor("xt16", [IN_F, B_CORE], F16, kind="ExternalInput")
    w = nc.dram_tensor("w", [K_DIM, OUT_F], F16, kind="ExternalInput")
    biasr = nc.dram_tensor("biasr", [128, OUT_F], F32, kind="ExternalInput")
    out = nc.dram_tensor("out", [B_CORE, OUT_F], F32, kind="ExternalOutput")

    with tile.TileContext(nc) as tc, ExitStack() as ctx:
        wpool = ctx.enter_context(tc.tile_pool(name="w", bufs=1))
        xpool = ctx.enter_context(tc.tile_pool(name="x", bufs=2))
        tpool = ctx.enter_context(tc.tile_pool(name="t", bufs=2))
        fpool = ctx.enter_context(tc.tile_pool(name="f", bufs=2))
        pspool = ctx.enter_context(tc.tile_pool(name="ps", bufs=8, space="PSUM"))

        # replicated bias, added during the PSUM->SBUF copy on VectorE
        bias_sb = wpool.tile([128, OUT_F], F32)
        nc.sync.dma_start(bias_sb[:], biasr[:, :])
        # resident weights [128, 48, OUT]; batched per-channel DMAs on the
        # idle HWDGE (sync) queue so they don't serialize ahead of x loads,
        # emitted in chunk-consumption order.
        w_sb = wpool.tile([128, N_CHUNK, OUT_F], F16)
        wr = w.rearrange("(c it p) o -> c p it o", p=128, it=NIT)
        for c in range(N_CH):
            nc.sync.dma_start(w_sb[:, c * NIT:(c + 1) * NIT, :], wr[c])
        # per-partition constant columns for activation bias operands
        bias_vals = sorted({-s0 / H for (s0, _) in SEGS}
                           | {-float(rr) for (_, n) in SEGS for rr in range(1, n)})
        cbias = wpool.tile([128, len(bias_vals)], F32)
        biasap = {}
        for j, val in enumerate(bias_vals):
            nc.vector.memset(cbias[:, j:j + 1], val)
            biasap[val] = cbias[:, j:j + 1]

        goff = 0
        for g, G in enumerate(GROUPS):
            c0, c1 = goff, goff + G
            goff += G
            # loads + tanh; every plane is [128, NIT, G], sliced per i-chunk
            xs = xpool.tile([128, NIT, G], F32, tag="xs")
            x6 = xpool.tile([128, NIT, G], F16, tag="x16")
            t = xs                             # tanh computed in place
            for it in range(NIT):
                nc.gpsimd.dma_start(xs[:, it, :], xt[it * 128:(it + 1) * 128, c0:c1])
                nc.gpsimd.dma_start(x6[:, it, :], xt16[it * 128:(it + 1) * 128, c0:c1])
                nc.scalar.activation(t[:, it, :], xs[:, it, :], AF.Tanh)

            planes = [x6]        # planes[c] : [128, NIT, G]; c=0 is x
            for (s0, n) in SEGS:
                capped = n * H + s0 < 1.0
                v = fpool.tile([128, NIT, G], F16, tag=f"v{s0}", name="v")
                v2 = fpool.tile([128, NIT, G], F16, tag=f"v2{s0}", name="v2")
                v3 = fpool.tile([128, NIT, G], F16, tag=f"v3{s0}", name="v3")
                rts = [(tpool.tile([128, NIT, G], F16, tag=f"r_{rr}", name="r"),
                        tpool.tile([128, NIT, G], F16, tag=f"r2_{rr}", name="r2"),
                        fpool.tile([128, NIT, G], F16, tag=f"r3{s0}_{rr}", name="r3"))
                       for rr in range(1, n)]
                for it in range(NIT):
                    v_ = v[:, it, :]
                    if capped:
                        # v = clamp(t/H - s0/H, 0, n) fully on the Vector engine
                        nc.vector.tensor_scalar(v_, t[:, it, :], 1.0 / H, -s0 / H,
                                                ALU.mult, ALU.add)
                        nc.vector.tensor_scalar(v_, v_, 0.0, float(n),
                                                ALU.max, ALU.min)
                    else:
                        # v = relu(t/H - s0/H) on the Scalar engine
                        nc.scalar.activation(v_, t[:, it, :], AF.Relu,
                                             bias=biasap[-s0 / H], scale=1.0 / H)
                    nc.vector.tensor_mul(v2[:, it, :], v_, v_)
                    nc.vector.tensor_mul(v3[:, it, :], v2[:, it, :], v_)
                    for rr, (r, r2, r3) in enumerate(rts, start=1):
                        nc.scalar.activation(r[:, it, :], v_, AF.Relu,
                                             bias=biasap[-float(rr)])
                        nc.vector.tensor_mul(r2[:, it, :], r[:, it, :], r[:, it, :])
                        nc.vector.tensor_mul(r3[:, it, :], r2[:, it, :], r[:, it, :])
                planes.append(v)
                planes.append(v2)
                planes.append(v3)
                for (_, _, r3) in rts:
                    planes.append(r3)

            assert len(planes) == N_CH
            for bs in range(G // 128):
                ps = pspool.tile([128, OUT_F], F32)
                for k in range(N_CHUNK):
                    c, it = divmod(k, NIT)
                    nc.tensor.matmul(
                        ps[:],
                        planes[c][:, it, bs * 128:(bs + 1) * 128],
                        w_sb[:, k, :],
                        start=(k == 0),
                        stop=(k == N_CHUNK - 1),
                    )
                o_sb = fpool.tile([128, OUT_F], F32, tag="osb")
                nc.vector.tensor_add(o_sb[:], ps[:], bias_sb[:])
                r0 = c0 + bs * 128
                nc.gpsimd.dma_start(out[r0:r0 + 128, :], o_sb[:])

    nc.finalize()
    return nc


def _get_nc():
    if "nc" not in _cache:
        _cache["nc"] = _build()
    return _cache["nc"]


# ---------------- public entry ----------------------------------------------
def _make_in_maps(x, coeffs, base_weight):
    W_dev, bias = _fold_weights(coeffs, base_weight)
    bias_rep = np.ascontiguousarray(
        np.broadcast_to(bias[None, :], (128, OUT_F)).astype(np.float32))

    xT = np.ascontiguousarray(x.T)               # [IN_F, B_TOT] f32
    xT16 = xT.astype(np.float16)

    in_maps = []
    for c in range(N_CORES):
        sl = slice(c * B_CORE, (c + 1) * B_CORE)
        in_maps.append({
            "xt": np.ascontiguousarray(xT[:, sl]),
            "xt16": np.ascontiguousarray(xT16[:, sl]),
            "w": W_dev,
            "biasr": bias_rep,
        })
    return in_maps


def kernel(x, coeffs, base_weight):
    x = np.asarray(x, np.float32)
    coeffs = np.asarray(coeffs, np.float32)
    base_weight = np.asarray(base_weight, np.float32)

    in_maps = _make_in_maps(x, coeffs, base_weight)
    nc = _get_nc()
    res = run_bass_kernel_spmd(nc, in_maps, core_ids=list(range(N_CORES)))
    out = np.concatenate([res.results[c]["out"] for c in range(N_CORES)], axis=0)
    return out.astype(np.float32), np.zeros((1,), np.float32)


# revision 35
# speedup vs baseline: 1.0265x; 1.0265x over previous
"""Trainium2 Bass kernel for nn_BSplineBasis (KAN-style B-spline layer).

reference:  t = tanh(x); basis = cubic B-spline basis of t over a uniform
            12-knot grid ([B, in, 8]); out = einsum('bin,ion->bo', basis,
            coeffs) + x @ base_weight;  kl = zeros(1).

Reformulation (exact in exact arithmetic): on t in [-1, 1] the 8 basis
functions span an 8-dim space of C^2 piecewise cubics with interior knots at
{-0.6,-0.2,0.2,0.6}. We represent them in a segmented clamped truncated-power
basis C(2,3): two segments starting at {-1.0, -0.2} covering {2, 3} grid
intervals; per segment, with v = clamp((t-s0)/h, 0, n), the channels are
{v, v^2, v^3} plus {relu(v-rr)^3 : rr=1..n-1}. That is 9 channels + a
constant. The constant folds to a per-output bias (added during the
PSUM->SBUF copy); the einsum + residual become one dense matmul over
K = (9+1)*512 = 5120 (9 spline channels + x), with weights host-folded from
coeffs/base_weight in float64.

All matmul operands are fp16 (measured 1.5e-3 rel L2 vs the fp32 reference;
channel values are O(1..27) and conversion coefficients are O(1), so fp16
rounding is not amplified). Accumulation is fp32 in PSUM.

Per core the pipeline processes batch-column groups (small first groups fill
the PE pipeline quickly): DMA x^T / x^T-fp16 -> tanh (Scalar engine, in
place) -> channels (Scalar relu + Vector clamps/multiplies, fp16) -> 40
accumulating matmuls per 128-row PSUM tile -> Vector copy+bias-add -> DMA.

Sharding: data-parallel over batch; each of the 8 cores handles 2048 rows.
Weights/bias are replicated (host-folded, ~5 MB fp16). x is pre-transposed
on host so the contraction dim lands on SBUF partitions.
"""

import numpy as np
from contextlib import ExitStack

import concourse.bass as bass
import concourse.mybir as mybir
import concourse.tile as tile
from concourse import bacc
from concourse.bass_utils import run_bass_kernel_spmd

F32 = mybir.dt.float32
F16 = mybir.dt.float16
AF = mybir.ActivationFunctionType
ALU = mybir.AluOpType

# ---------------- problem constants (hardcoded per harness contract) --------
B_TOT = 16384
IN_F = 512
OUT_F = 512
NUM_CTRL = 8
GRID_SIZE, ORDER = 5, 3
N_CORES = 8
B_CORE = B_TOT // N_CORES          # 2048
# batch-column group sizes per pipeline stage: small first groups fill the
# PE pipeline quickly; big groups amortize per-instruction overhead.
GROUPS = [128, 128, 256, 512, 512, 512]
assert sum(GROUPS) == B_CORE
H = 2.0 / GRID_SIZE                # 0.4 knot spacing
SEGS = [(-1.0, 2), (-0.2, 3)]   # (start, n_intervals)
N_CH = 1 + sum(3 + (n - 1) for (_, n) in SEGS)   # x + 9 spline channels = 10
K_DIM = N_CH * IN_F                # 5120
N_CHUNK = K_DIM // 128             # 40
NIT = IN_F // 128                  # 4 i-chunks per channel

_cache = {}


# ---------------- host-side spline algebra ----------------------------------
def _bspline_basis_np(x, grid, order):
    xe = x[..., None]
    b = ((xe >= grid[:-1]) & (xe < grid[1:])).astype(x.dtype)
    for k in range(1, order + 1):
        ld = grid[k:-1] - grid[:-k - 1]
        ld = np.where(ld == 0, 1.0, ld)
        rd = grid[k + 1:] - grid[1:-k]
        rd = np.where(rd == 0, 1.0, rd)
        b = ((xe - grid[:-k - 1]) / ld) * b[..., :-1] \
            + ((grid[k + 1:] - xe) / rd) * b[..., 1:]
    return b


def _seg_channels_np(t):
    chans = []
    for (s0, n) in SEGS:
        v = np.clip((t - s0) / H, 0.0, float(n))
        chans += [v, v * v, v * v * v]
        for rr in range(1, n):
            r = np.maximum(v - float(rr), 0.0)
            chans.append(r * r * r)
    return chans


def _conversion():
    """conv[1+9, 8]: basis_n(t) = conv[0,n]*1 + sum_c conv[1+c,n]*chan_c(t)."""
    grid = np.linspace(-1.0 - ORDER * H, 1.0 + ORDER * H,
                       GRID_SIZE + 2 * ORDER + 1)
    ts = np.linspace(-1.0, 1.0, 20001)
    basis = _bspline_basis_np(ts, grid, ORDER)          # [S, 8]
    A = np.stack([np.ones_like(ts)] + _seg_channels_np(ts), axis=1)
    conv, _, _, _ = np.linalg.lstsq(A, basis, rcond=None)
    fit_err = np.abs(A @ conv - basis).max()
    assert fit_err < 1e-10, f"spline conversion fit failed: {fit_err}"
    return conv                                          # [10, 8]


def _fold_weights(coeffs, base_weight):
    """Returns W_dev [K_DIM, OUT_F] fp16 (channel-major rows) and bias [OUT_F] f32."""
    conv = _conversion()
    c64 = coeffs.astype(np.float64)
    bias = np.einsum('n,ion->o', conv[0], c64).astype(np.float32)
    W = np.empty((N_CH, IN_F, OUT_F), np.float64)
    W[0] = base_weight.astype(np.float64)
    for c in range(1, N_CH):
        W[c] = np.einsum('n,ion->io', conv[c], c64)
    return W.reshape(K_DIM, OUT_F).astype(np.float16), bias


# ---------------- device kernel ---------------------------------------------
def _build():
    nc = bacc.Bacc()
    xt = nc.dram_tensor("xt", [IN_F, B_CORE], F32, kind="ExternalInput")
    xt16 = nc.dram_tensor("xt16", [IN_F, B_CORE], F16, kind="ExternalInput")
    w = nc.dram_tensor("w", [K_DIM, OUT_F], F16, kind="ExternalInput")
    biasr = nc.dram_tensor("biasr", [128, OUT_F], F32, kind="ExternalInput")
    out = nc.dram_tensor("out", [B_CORE, OUT_F], F32, kind="ExternalOutput")

    with tile.TileContext(nc) as tc, ExitStack() as ctx:
        wpool = ctx.enter_context(tc.tile_pool(name="w", bufs=1))
        xpool = ctx.enter_context(tc.tile_pool(name="x", bufs=2))
        tpool = ctx.enter_context(tc.tile_pool(name="t", bufs=2))
        fpool = ctx.enter_context(tc.tile_pool(name="f", bufs=2))
        pspool = ctx.enter_context(tc.tile_pool(name="ps", bufs=8, space="PSUM"))

        # replicated bias, added during the PSUM->SBUF copy on VectorE
        bias_sb = wpool.tile([128, OUT_F], F32)
        nc.sync.dma_start(bias_sb[:], biasr[:, :])
        # resident weights [128, 48, OUT]; batched per-channel DMAs on the
        # idle HWDGE (sync) queue so they don't serialize ahead of x loads,
        # emitted in chunk-consumption order.
        w_sb = wpool.tile([128, N_CHUNK, OUT_F], F16)
        wr = w.rearrange("(c it p) o -> c p it o", p=128, it=NIT)
        for c in range(N_CH):
            nc.sync.dma_start(w_sb[:, c * NIT:(c + 1) * NIT, :], wr[c])
        # per-partition constant columns for activation bias operands
        bias_vals = sorted({-s0 / H for (s0, _) in SEGS}
                           | {-float(rr) for (_, n) in SEGS for rr in range(1, n)})
        cbias = wpool.tile([128, len(bias_vals)], F32)
        biasap = {}
        for j, val in enumerate(bias_vals):
            nc.vector.memset(cbias[:, j:j + 1], val)
            biasap[val] = cbias[:, j:j + 1]

        goff = 0
        for g, G in enumerate(GROUPS):
            c0, c1 = goff, goff + G
            goff += G
            # loads + tanh; every plane is [128, NIT, G], sliced per i-chunk
            xs = xpool.tile([128, NIT, G], F32, tag="xs")
            x6 = xpool.tile([128, NIT, G], F16, tag="x16")
            t = xs                             # tanh computed in place
            for it in range(NIT):
                nc.gpsimd.dma_start(xs[:, it, :], xt[it * 128:(it + 1) * 128, c0:c1])
                nc.gpsimd.dma_start(x6[:, it, :], xt16[it * 128:(it + 1) * 128, c0:c1])
                nc.scalar.activation(t[:, it, :], xs[:, it, :], AF.Tanh)

            planes = [x6]        # planes[c] : [128, NIT, G]; c=0 is x
            for (s0, n) in SEGS:
                capped = n * H + s0 < 1.0
                v = fpool.tile([128, NIT, G], F16, tag=f"v{s0}", name="v")
                v2 = fpool.tile([128, NIT, G], F16, tag=f"v2{s0}", name="v2")
                v3 = fpool.tile([128, NIT, G], F16, tag=f"v3{s0}", name="v3")
                rts = [(tpool.tile([128, NIT, G], F16, tag=f"r_{rr}", name="r"),
                        tpool.tile([128, NIT, G], F16, tag=f"r2_{rr}", name="r2"),
                        fpool.tile([128, NIT, G], F16, tag=f"r3{s0}_{rr}", name="r3"))
                       for rr in range(1, n)]
                for it in range(NIT):
                    v_ = v[:, it, :]
                    if capped:
                        # v = clamp(t/H - s0/H, 0, n) fully on the Vector engine
                        nc.vector.tensor_scalar(v_, t[:, it, :], 1.0 / H, -s0 / H,
                                                ALU.mult, ALU.add)
                        nc.vector.tensor_scalar(v_, v_, 0.0, float(n),
                                                ALU.max, ALU.min)
                    else:
                        # v = relu(t/H - s0/H) on the Scalar engine
                        nc.scalar.activation(v_, t[:, it, :], AF.Relu,
                                             bias=biasap[-s0 / H], scale=1.0 / H)
                    nc.vector.tensor_mul(v2[:, it, :], v_, v_)
                    nc.vector.tensor_mul(v3[:, it, :], v2[:, it, :], v_)
                    for rr, (r, r2, r3) in enumerate(rts, start=1):
                        nc.scalar.activation(r[:, it, :], v_, AF.Relu,
                                             bias=biasap[-float(rr)])
                        nc.vector.tensor_mul(r2[:, it, :], r[:, it, :], r[:, it, :])
                        nc.vector.tensor_mul(r3[:, it, :], r2[:, it, :], r[:, it, :])
                planes.append(v)
                planes.append(v2)
                planes.append(v3)
                for (_, _, r3) in rts:
                    planes.append(r3)

            assert len(planes) == N_CH
            for bs in range(G // 128):
                ps = pspool.tile([128, OUT_F], F32)
                for k in range(N_CHUNK):
                    c, it = divmod(k, NIT)
                    nc.tensor.matmul(
                        ps[:],
                        planes[c][:, it, bs * 128:(bs + 1) * 128],
                        w_sb[:, k, :],
                        start=(k == 0),
                        stop=(k == N_CHUNK - 1),
                    )
                o_sb = fpool.tile([128, OUT_F], F32, tag="osb")
                nc.vector.tensor_add(o_sb[:], ps[:], bias_sb[:])
                r0 = c0 + bs * 128
                # alternate output DMAs across two queues so the final
                # end-of-kernel DMA drain doesn't serialize on one queue
                oeng = nc.gpsimd if (r0 // 128) % 2 == 0 else nc.scalar
                oeng.dma_start(out[r0:r0 + 128, :], o_sb[:])

    nc.finalize()
    return nc


def _get_nc():
    if "nc" not in _cache:
        _cache["nc"] = _build()
    return _cache["nc"]


# ---------------- public entry ----------------------------------------------
def _make_in_maps(x, coeffs, base_weight):
    W_dev, bias = _fold_weights(coeffs, base_weight)
    bias_rep = np.ascontiguousarray(
        np.broadcast_to(bias[None, :], (128, OUT_F)).astype(np.float32))

    xT = np.ascontiguousarray(x.T)               # [IN_F, B_TOT] f32
    xT16 = xT.astype(np.float16)

    in_maps = []
    for c in range(N_CORES):
        sl = slice(c * B_CORE, (c + 1) * B_CORE)
        in_maps.append({
            "xt": np.ascontiguousarray(xT[:, sl]),
            "xt16": np.ascontiguousarray(xT16[:, sl]),
            "w": W_dev,
            "biasr": bias_rep,
        })
    return in_maps


def kernel(x, coeffs, base_weight):
    x = np.asarray(x, np.float32)
    coeffs = np.asarray(coeffs, np.float32)
    base_weight = np.asarray(base_weight, np.float32)

    in_maps = _make_in_maps(x, coeffs, base_weight)
    nc = _get_nc()
    res = run_bass_kernel_spmd(nc, in_maps, core_ids=list(range(N_CORES)))
    out = np.concatenate([res.results[c]["out"] for c in range(N_CORES)], axis=0)
    return out.astype(np.float32), np.zeros((1,), np.float32)
